# revision 1
# baseline (speedup 1.0000x reference)
"""BitAttention TRN2 kernel: 8-core tensor-parallel (head-split).

Sharding: core c owns heads (2c, 2c+1) = channels [256c, 256c+256) of the
q/k/v projections (column split) and of the output channels of out_proj
(column split).  Attention is fully local to a core; one AllGather of the
(bf16-triplet-split) attention output feeds the out-projection.  The three
global reductions in the quantizers (mean|w|, max/min of q/k/v pre-acts,
max/min of out_proj pre-acts) are tiny AllReduces.

Precision strategy (validated numerically: rel-L2 ~3e-4 vs fp64 reference):
  - x / attn_out are split into bf16 triplets (hi+lo+lo2 ~ 27 mantissa bits);
    ternary weights are bf16-exact, so 3 bf16 passes == fp32-grade matmul at
    3/4 the cost of the PE's 4-pass true-fp32 mode.
  - quantized activations are small integers (|n|<=255, bf16/fp16-exact):
    scores = n_q.n_k in ONE bf16 pass, exactly integer.
  - softmax: row-max (exact integer, from a layout-1 scores pass) is folded
    into the layout-2 scores PSUM as a rank-1 fp32r matmul update; exp runs on
    ACT straight out of PSUM into fp16; attn@v and the denominator both
    consume the SAME fp16 exp values, so LUT/rounding errors cancel in the
    softmax ratio (softmax is shift-invariant, so the fp32r truncation of the
    max is harmless).
"""

import numpy as np
import ml_dtypes

DIM = 2048
NCORES = 8
CH = DIM // NCORES          # 256 channels per core
B, S = 2, 2048
T = B * S                   # 4096 tokens
KC = DIM // 128             # 16 contraction chunks
TT = 512                    # token tile
NTT = T // TT
MAGIC = float(1.5 * 2 ** 23)      # fp32 round-to-nearest-even via add/sub
F32MAX = float(np.finfo(np.float32).max)

_cache = {}


def _build(single=False, stop_after=None):
    import concourse.bass as bass  # noqa: F401
    import concourse.mybir as mybir
    import concourse.tile as tile
    from concourse import bacc
    from concourse.bass_isa import ReduceOp
    from concourse.masks import make_identity

    f32 = mybir.dt.float32
    f32r = mybir.dt.float32r
    bf16 = mybir.dt.bfloat16
    f16 = mybir.dt.float16
    AX = mybir.AxisListType.X
    OP = mybir.AluOpType
    AF = mybir.ActivationFunctionType

    _ORDER = ["W", "Q", "C2", "S", "G", "O"]

    def _go(ph):
        return stop_after is None or _ORDER.index(ph) <= _ORDER.index(stop_after)

    nc = bacc.Bacc("TRN2", target_bir_lowering=False, debug=False,
                   num_devices=1 if single else NCORES)

    def collective(kind, op, ins, outs):
        if single:
            # TimelineSim mode: stand in for the collective with a DMA copy
            # (approximate wire time is accounted for separately).
            if kind == "AllGather":
                for r in range(NCORES):
                    nc.sync.dma_start(outs[0].tensor.ap()[r * 3 * CH:(r + 1) * 3 * CH, :],
                                      ins[0])
            else:
                nc.sync.dma_start(outs[0], ins[0])
        else:
            nc.gpsimd.collective_compute(kind, op, replica_groups=[list(range(NCORES))],
                                         ins=[ins[0]], outs=[outs[0]])

    def nrecip(pool, out_ap, d_ap, nm, shape=None):
        """out = 1/d with one Newton refinement on top of DVE reciprocal."""
        shape = shape or [d_ap.shape[0], d_ap.shape[-1]]
        g0 = pool.tile(shape, f32, tag=f"nr0_{shape[-1]}", name=f"g0_{nm}")
        t = pool.tile(shape, f32, tag=f"nr1_{shape[-1]}", name=f"t_{nm}")
        u = pool.tile(shape, f32, tag=f"nr2_{shape[-1]}", name=f"u_{nm}")
        nc.vector.reciprocal(g0[:], d_ap)
        nc.vector.tensor_tensor(out=t[:], in0=d_ap, in1=g0[:], op=OP.mult)
        nc.vector.tensor_scalar(out=t[:], in0=t[:], scalar1=1.0, scalar2=None,
                                op0=OP.subtract)
        nc.vector.tensor_tensor(out=u[:], in0=g0[:], in1=t[:], op=OP.mult)
        nc.vector.tensor_tensor(out=out_ap, in0=g0[:], in1=u[:], op=OP.subtract)

    # ---------------- I/O ----------------
    xhi = nc.dram_tensor("xhi", [DIM, T], bf16, kind="ExternalInput").ap()
    xlo = nc.dram_tensor("xlo", [DIM, T], bf16, kind="ExternalInput").ap()
    xlo2 = nc.dram_tensor("xlo2", [DIM, T], bf16, kind="ExternalInput").ap()
    wT = {p: nc.dram_tensor(f"w{p}", [DIM, CH], f32, kind="ExternalInput").ap()
          for p in "qkvo"}
    bias = {p: nc.dram_tensor(f"b{p}", [CH], f32, kind="ExternalInput").ap()
            for p in "qkvo"}
    sca = {p: nc.dram_tensor(f"s{p}", [1, 1], f32, kind="ExternalInput").ap()
           for p in "qkvo"}
    o_out = nc.dram_tensor("o_out", [CH, T], f32, kind="ExternalOutput").ap()

    xv = {0: xhi.rearrange("(c p) t -> p c t", p=128),
          1: xlo.rearrange("(c p) t -> p c t", p=128),
          2: xlo2.rearrange("(c p) t -> p c t", p=128)}
    wTv = {p: wT[p].rearrange("(c p) o -> p c o", p=128) for p in "qkvo"}
    bv = {p: bias[p].rearrange("(m p) -> p m", p=128) for p in "qkvo"}
    o_outv = o_out.rearrange("(m p) t -> p m t", p=128)

    RG = [list(range(NCORES))]

    with tile.TileContext(nc) as tc:
        with tc.tile_pool(name="persist", bufs=1) as P, \
             tc.tile_pool(name="dram", bufs=1, space="DRAM") as D:

            # ---- persistent arenas (~58KB/partition) ----
            wo_ter = P.tile([128, KC, CH], bf16, name="wo_ter")
            nqT = P.tile([128, 2, T], bf16, name="nqT")      # [d, head, tok]
            nkT = P.tile([128, 2, T], bf16, name="nkT")
            n_v = P.tile([128, T // 128, CH], f16, name="n_v")  # [tok%128, tc, ch]
            ident32 = P.tile([128, 128], f32, name="ident32")
            ident16 = P.tile([128, 128], f16, name="ident16")
            ones16 = P.tile([128, 1], f16, name="ones16")
            ones_r = P.tile([1, 128], f32, name="ones_r")
            scal = P.tile([1, 16], f32, name="scal")         # partition-0 scalars
            scalB = P.tile([128, 4], f32, name="scalB")      # broadcast scalars
            stat_q = P.tile([128, 8], f32, name="stat_q")    # qkv+o max/negmin
            wsum = P.tile([128, 4], f32, name="wsum")
            wsum2 = P.tile([128, 4], f32, name="wsum2")
            c255 = P.tile([1, 1], f32, name="c255")

            make_identity(nc, ident32[:])
            make_identity(nc, ident16[:])
            nc.vector.memset(ones16[:], 1.0)
            nc.vector.memset(ones_r[:], 1.0)
            nc.vector.memset(c255[:], 255.0)
            nc.vector.memset(stat_q[:], -F32MAX)

            # ---- dram scratch ----
            pre_d = {p: D.tile([2, 128, T], f32, name=f"pre_{p}") for p in "qkv"}
            cc1_in = D.tile([1, 4], f32, name="cc1_in")
            cc1_out = D.tile([1, 4], f32, name="cc1_out", addr_space="Shared")
            cc2_in = D.tile([1, 6], f32, name="cc2_in")
            cc2_out = D.tile([1, 6], f32, name="cc2_out", addr_space="Shared")
            cc3_in = D.tile([1, 2], f32, name="cc3_in")
            cc3_out = D.tile([1, 2], f32, name="cc3_out", addr_space="Shared")
            ag_in = D.tile([3 * CH, T], bf16, name="ag_in")
            ag_out = D.tile([3 * CH * NCORES, T], bf16, name="ag_out",
                            addr_space="Local" if single else "Shared")
            ag_outv = ag_out[:].rearrange("(c p) t -> p c t", p=128)  # [128,48,T]

            with tc.tile_pool(name="wter", bufs=1) as WT:
                wq_ter = {p: WT.tile([128, KC, CH], bf16, name=f"wter_{p}")
                          for p in "qkv"}
                wq_ter["o"] = wo_ter

                # ============ Phase W: weight ternarization ============
                with tc.tile_pool(name="wstage", bufs=1) as WS:
                    s_b = WS.tile([128, 4], f32, name="s_b")
                    for pi, p in enumerate("qkvo"):
                        s_sb = WS.tile([1, 1], f32, tag="ssb", bufs=4,
                                       name=f"ssb_{p}")
                        nc.sync.dma_start(s_sb[:], sca[p])
                        nc.gpsimd.partition_broadcast(s_b[:, pi:pi + 1], s_sb[:])
                    for pi, p in enumerate("qkvo"):
                        wf = WS.tile([128, KC, CH], f32, tag=f"wf{pi % 2}",
                                     name=f"wf_{p}")
                        nc.sync.dma_start(wf[:], wTv[p])
                        wabs = WS.tile([128, KC, CH], f32, tag="wabs",
                                       name=f"wabs_{p}")
                        nc.vector.tensor_scalar_mul(wabs[:], wf[:], s_b[:, pi:pi + 1])
                        wl1 = WS.tile([128, KC], f32, tag="wl1", name=f"wl1_{p}")
                        nc.vector.tensor_reduce(
                            out=wl1[:], in_=wabs[:],
                            axis=AX, op=OP.add, apply_absolute_value=True)
                        nc.vector.tensor_reduce(
                            out=wsum[:, pi:pi + 1], in_=wl1[:],
                            axis=AX, op=OP.add)
                    nc.gpsimd.partition_all_reduce(wsum2[:], wsum[:], channels=128,
                                                   reduce_op=ReduceOp.add)
                    nc.sync.dma_start(cc1_in[:], wsum2[0:1, 0:4])
                    collective("AllReduce", OP.add, [cc1_in[:].opt()],
                               [cc1_out[:].opt()])
                    nc.sync.dma_start(scal[:, 0:4], cc1_out[:])
                    thr = WS.tile([1, 4], f32, name="thr")
                    nthr = WS.tile([1, 4], f32, name="nthr")
                    nc.vector.tensor_scalar_mul(thr[:], scal[:, 0:4],
                                                0.7 / (DIM * DIM))
                    nc.vector.tensor_scalar_mul(nthr[:], thr[:], -1.0)
                    thr_b = WS.tile([128, 4], f32, name="thr_b")
                    nthr_b = WS.tile([128, 4], f32, name="nthr_b")
                    nc.gpsimd.partition_broadcast(thr_b[:], thr[:])
                    nc.gpsimd.partition_broadcast(nthr_b[:], nthr[:])
                    for pi, p in enumerate("qkvo"):
                        wf = WS.tile([128, KC, CH], f32, tag=f"wf{pi % 2}",
                                     name=f"wf2_{p}")
                        nc.sync.dma_start(wf[:], wTv[p])
                        gt = WS.tile([128, KC, CH], f32, tag="gt", name=f"gt_{p}")
                        lt = WS.tile([128, KC, CH], f32, tag="lt", name=f"lt_{p}")
                        # (w*s > thr) and (w*s < -thr) fused: mult then compare
                        nc.vector.tensor_scalar(out=gt[:], in0=wf[:],
                                                scalar1=s_b[:, pi:pi + 1],
                                                scalar2=thr_b[:, pi:pi + 1],
                                                op0=OP.mult, op1=OP.is_gt)
                        nc.vector.tensor_scalar(out=lt[:], in0=wf[:],
                                                scalar1=s_b[:, pi:pi + 1],
                                                scalar2=nthr_b[:, pi:pi + 1],
                                                op0=OP.mult, op1=OP.is_lt)
                        nc.vector.tensor_tensor(out=wq_ter[p][:], in0=gt[:],
                                                in1=lt[:], op=OP.subtract)

                # ============ Phase Q: QKV projections (bf16 x3) ============
                _doQ = _go("Q")
                with tc.tile_pool(name="xstage", bufs=2) as XS, \
                     tc.tile_pool(name="qpsum", bufs=4, space="PSUM") as QP, \
                     tc.tile_pool(name="qout", bufs=3) as QO:
                    bsb = QO.tile([128, 3, 2], f32, bufs=1, name="bsb")
                    for pi, p in enumerate("qkv"):
                        nc.sync.dma_start(bsb[:, pi, :], bv[p])
                    for tt in range(NTT if _doQ else 0):
                        xs = {}
                        for pas in range(3):
                            xt = XS.tile([128, KC, TT], bf16, tag=f"x{pas}",
                                         name=f"x{pas}_{tt}")
                            nc.sync.dma_start(
                                xt[:], xv[pas][:, :, tt * TT:(tt + 1) * TT])
                            xs[pas] = xt
                        for pi, p in enumerate("qkv"):
                            for m in range(2):
                                ps = QP.tile([128, TT], f32, tag="qp",
                                             name=f"qp{p}{m}{tt}")
                                n = 0
                                for pas in range(3):
                                    for kc in range(KC):
                                        nc.tensor.matmul(
                                            ps[:],
                                            wq_ter[p][:, kc, m * 128:(m + 1) * 128],
                                            xs[pas][:, kc, :],
                                            start=(n == 0), stop=(n == 3 * KC - 1))
                                        n += 1
                                pre = QO.tile([128, TT], f32, tag="pre",
                                              name=f"pre{p}{m}{tt}")
                                nc.scalar.activation(pre[:], ps[:], AF.Identity,
                                                     bias=bsb[:, pi, m:m + 1],
                                                     scale=1.0)
                                six = 2 * pi
                                tmx = QO.tile([128, 2], f32, tag="tmx",
                                              name=f"tmx{p}{m}{tt}")
                                nc.vector.tensor_reduce(out=tmx[:, 0:1], in_=pre[:],
                                                        axis=AX, op=OP.max)
                                nc.vector.tensor_reduce(out=tmx[:, 1:2], in_=pre[:],
                                                        axis=AX, op=OP.min,
                                                        negate=True)
                                nc.vector.tensor_tensor(out=stat_q[:, six:six + 2],
                                                        in0=stat_q[:, six:six + 2],
                                                        in1=tmx[:], op=OP.max)
                                nc.sync.dma_start(
                                    pre_d[p][m, :, tt * TT:(tt + 1) * TT], pre[:])

            _doC2 = _go("C2")
            # ---- global max/min AllReduce + quantize q/k/v ----
            stat2 = P.tile([128, 6], f32, name="stat2")
            nc.gpsimd.partition_all_reduce(stat2[:], stat_q[:, 0:6], channels=128,
                                           reduce_op=ReduceOp.max)
            nc.sync.dma_start(cc2_in[:], stat2[0:1, 0:6])
            collective("AllReduce", OP.max, [cc2_in[:].opt()], [cc2_out[:].opt()])
            nc.sync.dma_start(scal[:, 4:10], cc2_out[:])
            with tc.tile_pool(name="qquant", bufs=2) as QQ, \
                 tc.tile_pool(name="qqpsum", bufs=2, space="PSUM") as TP:
                nvT = QQ.tile([128, 2, T], f16, bufs=1, name="nvT")
                scl = QQ.tile([1, 3], f32, bufs=1, name="scl")
                for pi in range(3):
                    df = QQ.tile([1, 1], f32, tag="df", name=f"df{pi}")
                    nc.vector.tensor_tensor(out=df[:],
                                            in0=scal[:, 4 + 2 * pi:5 + 2 * pi],
                                            in1=scal[:, 5 + 2 * pi:6 + 2 * pi],
                                            op=OP.add)
                    rcp = QQ.tile([1, 1], f32, tag="rcp", name=f"rcp{pi}")
                    nrecip(QQ, rcp[:], df[:], f"rscl{pi}")
                    nc.vector.tensor_scalar_mul(scl[:, pi:pi + 1], rcp[:], 255.0)
                sclB = QQ.tile([128, 3], f32, bufs=1, name="sclB")
                nc.gpsimd.partition_broadcast(sclB[:], scl[:])
                # Dexp = 1/(scale_q*scale_k*sqrt(128)); broadcast to scalB[:,0]
                tmp = QQ.tile([1, 1], f32, bufs=1, name="tmpd")
                nc.vector.tensor_tensor(out=tmp[:], in0=scl[:, 0:1], in1=scl[:, 1:2],
                                        op=OP.mult)
                nc.vector.tensor_scalar_mul(tmp[:], tmp[:], float(np.sqrt(128.0)))
                dexp = QQ.tile([1, 1], f32, bufs=1, name="dexp")
                nrecip(QQ, dexp[:], tmp[:], "rdexp")
                nc.gpsimd.partition_broadcast(scalB[:, 0:1], dexp[:])
                nc.gpsimd.partition_broadcast(scalB[:, 1:2], scl[:, 2:3])  # scale_v
                for pi, p in enumerate("qkv" if _doC2 else ""):
                    for m in range(2):
                        st = QQ.tile([128, T], f32, tag="qst", name=f"qst{p}{m}")
                        nc.sync.dma_start(st[:], pre_d[p][m])
                        t1 = QQ.tile([128, T], f32, tag="qt1", name=f"qt1{p}{m}")
                        nc.vector.tensor_scalar(out=t1[:], in0=st[:],
                                                scalar1=sclB[:, pi:pi + 1],
                                                scalar2=MAGIC, op0=OP.mult,
                                                op1=OP.add)
                        dst = {"q": nqT, "k": nkT, "v": nvT}[p]
                        nc.vector.tensor_scalar(out=dst[:, m, :], in0=t1[:],
                                                scalar1=MAGIC, scalar2=None,
                                                op0=OP.subtract)
                for m in range(2):
                    for tch in range(T // 128):
                        pst = TP.tile([128, 128], f16, tag="tp", name=f"tp{m}_{tch}")
                        nc.tensor.transpose(pst[:],
                                            nvT[:, m, tch * 128:(tch + 1) * 128],
                                            ident16[:])
                        nc.vector.tensor_copy(n_v[:, tch, m * 128:(m + 1) * 128],
                                              pst[:])

            # ============ Phase S: attention, per (head, batch) ============
            _doS = _go("S")
            with tc.tile_pool(name="aout", bufs=1) as AO:
                aoutT = AO.tile([128, 2, T], f32, name="aoutT")
                with tc.tile_pool(name="sexp", bufs=1) as SE, \
                     tc.tile_pool(name="ssm", bufs=2) as SP, \
                     tc.tile_pool(name="spp", bufs=4, space="PSUM") as PP, \
                     tc.tile_pool(name="spd", bufs=2, space="PSUM") as PD:
                    for h in range(2 if _doS else 0):
                        for b in range(2):
                            expT = SE.tile([128, KC, S], f16, tag="expT",
                                           name=f"expT{h}{b}")
                            negm = SP.tile([128, 16], f32, tag="negm",
                                           name=f"negm{h}{b}")
                            # S1: per-q-row max of integer scores (layout [q,k])
                            for qc in range(16):
                                m4 = SP.tile([128, 4], f32, tag="m4",
                                             name=f"m4_{h}{b}{qc}")
                                q0 = b * S + qc * 128
                                for kt in range(4):
                                    k0 = b * S + kt * 512
                                    pss = PP.tile([128, 512], f32, tag="b1",
                                                  name=f"ss{h}{b}{qc}{kt}")
                                    nc.tensor.matmul(pss[:], nqT[:, h, q0:q0 + 128],
                                                     nkT[:, h, k0:k0 + 512],
                                                     start=True, stop=True)
                                    nc.vector.tensor_reduce(out=m4[:, kt:kt + 1],
                                                            in_=pss[:], axis=AX,
                                                            op=OP.max)
                                nc.vector.tensor_reduce(out=negm[:, qc:qc + 1],
                                                        in_=m4[:], axis=AX, op=OP.max,
                                                        negate=True)
                            # S2: negm [128,16] -> one q-ordered row [1, 2048]
                            negm_pad = SP.tile([128, 128], f32, tag="npad",
                                               name=f"npad{h}{b}")
                            nc.vector.memset(negm_pad[:], 0.0)
                            nc.vector.tensor_copy(negm_pad[:, 0:16], negm[:])
                            pnt = PP.tile([128, 128], f32, tag="b1", name=f"pnt{h}{b}")
                            nc.tensor.transpose(pnt[:], negm_pad[:], ident32[:])
                            negmT = SP.tile([16, 128], f32, tag="negmT",
                                            name=f"negmT{h}{b}")
                            nc.vector.tensor_copy(negmT[:], pnt[0:16, :])
                            negm_row = SP.tile([1, S], f32, tag="nrow",
                                               name=f"nrow{h}{b}")
                            nc.sync.dma_start(negm_row[:], negmT[:])
                            # S3: scores^T + rank-1(-max) -> exp(fp16) -> den, attn@v
                            for qt in range(4):
                                qs = b * S + qt * 512
                                for kc in range(KC):
                                    k0 = b * S + kc * 128
                                    pT = PP.tile([128, 512], f32, tag="b1",
                                                 name=f"pT{h}{b}{qt}{kc}")
                                    nc.tensor.matmul(pT[:], nkT[:, h, k0:k0 + 128],
                                                     nqT[:, h, qs:qs + 512],
                                                     start=True, stop=False)
                                    nc.tensor.matmul(
                                        pT[:], ones_r[:].bitcast(f32r),
                                        negm_row[:].bitcast(f32r)[:,
                                            qt * 512:(qt + 1) * 512],
                                        start=False, stop=True,
                                        skip_group_check=True)
                                    nc.scalar.activation(
                                        expT[:, kc, qt * 512:(qt + 1) * 512], pT[:],
                                        AF.Exp, bias=0.0, scale=scalB[:, 0:1])
                                pden = PD.tile([1, 512], f32, tag="den",
                                               name=f"den{h}{b}{qt}")
                                pout = PP.tile([128, 512], f32, tag="b1",
                                               name=f"po{h}{b}{qt}")
                                for kc in range(KC):
                                    e_ap = expT[:, kc, qt * 512:(qt + 1) * 512]
                                    nc.tensor.matmul(pden[:], ones16[:], e_ap,
                                                     start=(kc == 0),
                                                     stop=(kc == KC - 1))
                                    nc.tensor.matmul(
                                        pout[:],
                                        n_v[:, b * 16 + kc, h * 128:(h + 1) * 128],
                                        e_ap, start=(kc == 0), stop=(kc == KC - 1))
                                dsb = SP.tile([1, 512], f32, tag="dsb",
                                              name=f"dsb{h}{b}{qt}")
                                nc.vector.tensor_scalar(out=dsb[:], in0=pden[:],
                                                        scalar1=scalB[0:1, 1:2],
                                                        scalar2=None, op0=OP.mult)
                                grow = SP.tile([1, 512], f32, tag="grow",
                                               name=f"grow{h}{b}{qt}")
                                nrecip(SP, grow[:], dsb[:], f"rg{h}{b}{qt}")
                                gb = SP.tile([128, 512], f32, tag="gb",
                                             name=f"gb{h}{b}{qt}")
                                nc.gpsimd.partition_broadcast(gb[:], grow[:])
                                nc.vector.tensor_tensor(
                                    out=aoutT[:, h, qs:qs + 512],
                                    in0=pout[:], in1=gb[:], op=OP.mult)

                # ============ Phase O part 1: split + AllGather ============
                _doG = _go("G")
                with tc.tile_pool(name="osplit", bufs=1) as OS:
                    ahi = OS.tile([128, 2, T], bf16, name="ahi")
                    alo = OS.tile([128, 2, T], bf16, name="alo")
                    alo2 = OS.tile([128, 2, T], bf16, name="alo2")
                    atmp = OS.tile([128, 2, T], f32, name="atmp")
                    nc.vector.tensor_copy(ahi[:], aoutT[:])
                    nc.vector.tensor_tensor(out=alo[:], in0=aoutT[:], in1=ahi[:],
                                            op=OP.subtract)
                    nc.vector.tensor_tensor(out=atmp[:], in0=aoutT[:], in1=ahi[:],
                                            op=OP.subtract)
                    nc.vector.tensor_tensor(out=alo2[:], in0=atmp[:], in1=alo[:],
                                            op=OP.subtract)
                    agv = ag_in[:].rearrange("(s m p) t -> p s m t", p=128, s=3)
                    nc.sync.dma_start(agv[:, 0], ahi[:])
                    nc.sync.dma_start(agv[:, 1], alo[:])
                    nc.sync.dma_start(agv[:, 2], alo2[:])
            collective("AllGather", OP.bypass, [ag_in[:].opt()], [ag_out[:].opt()])

            # ============ Phase O part 2: out-projection (bf16 x3) ============
            _doO = _go("O")
            with tc.tile_pool(name="oarena", bufs=1) as OA:
                opre = OA.tile([128, 2, T], f32, name="opre")
                with tc.tile_pool(name="ostage", bufs=2) as OG, \
                     tc.tile_pool(name="opsum", bufs=4, space="PSUM") as OPP:
                    bosb = OG.tile([128, 2], f32, bufs=1, name="bosb")
                    nc.sync.dma_start(bosb[:], bv["o"])
                    for tt in range(NTT if _doO else 0):
                        ast = OG.tile([128, 48, TT], bf16, tag="ast", name=f"ast{tt}")
                        nc.sync.dma_start(ast[:],
                                          ag_outv[:, :, tt * TT:(tt + 1) * TT])
                        for m in range(2):
                            ps = OPP.tile([128, TT], f32, tag="op", name=f"op{m}{tt}")
                            n = 0
                            for pas in range(3):
                                for kc in range(KC):
                                    r, sub = kc // 2, kc % 2
                                    c = r * 6 + pas * 2 + sub
                                    nc.tensor.matmul(
                                        ps[:], wo_ter[:, kc, m * 128:(m + 1) * 128],
                                        ast[:, c, :],
                                        start=(n == 0), stop=(n == 3 * KC - 1))
                                    n += 1
                            nc.scalar.activation(opre[:, m, tt * TT:(tt + 1) * TT],
                                                 ps[:], AF.Identity,
                                                 bias=bosb[:, m:m + 1], scale=1.0)
                            tmx = OG.tile([128, 2], f32, tag="otmx",
                                          name=f"otmx{m}{tt}")
                            nc.vector.tensor_reduce(
                                out=tmx[:, 0:1],
                                in_=opre[:, m, tt * TT:(tt + 1) * TT],
                                axis=AX, op=OP.max)
                            nc.vector.tensor_reduce(
                                out=tmx[:, 1:2],
                                in_=opre[:, m, tt * TT:(tt + 1) * TT],
                                axis=AX, op=OP.min, negate=True)
                            nc.vector.tensor_tensor(out=stat_q[:, 6:8],
                                                    in0=stat_q[:, 6:8], in1=tmx[:],
                                                    op=OP.max)
                # ---- final quantization ----
                stat3 = P.tile([128, 2], f32, name="stat3")
                nc.gpsimd.partition_all_reduce(stat3[:], stat_q[:, 6:8],
                                               channels=128, reduce_op=ReduceOp.max)
                nc.sync.dma_start(cc3_in[:], stat3[0:1, 0:2])
                collective("AllReduce", OP.max, [cc3_in[:].opt()],
                           [cc3_out[:].opt()])
                nc.sync.dma_start(scal[:, 10:12], cc3_out[:])
                with tc.tile_pool(name="oquant", bufs=2) as OQ:
                    df = OQ.tile([1, 1], f32, bufs=1, name="odf")
                    nc.vector.tensor_tensor(out=df[:], in0=scal[:, 10:11],
                                            in1=scal[:, 11:12], op=OP.add)
                    oscl = OQ.tile([1, 1], f32, bufs=1, name="oscl")
                    orcp = OQ.tile([1, 1], f32, bufs=1, name="orcp")
                    nrecip(OQ, orcp[:], df[:], "rorcp")
                    nc.vector.tensor_scalar_mul(oscl[:], orcp[:], 255.0)
                    oinv = OQ.tile([1, 1], f32, bufs=1, name="oinv")
                    nrecip(OQ, oinv[:], oscl[:], "roinv")
                    osclB = OQ.tile([128, 1], f32, bufs=1, name="osclB")
                    oinvB = OQ.tile([128, 1], f32, bufs=1, name="oinvB")
                    nc.gpsimd.partition_broadcast(osclB[:], oscl[:])
                    nc.gpsimd.partition_broadcast(oinvB[:], oinv[:])
                    for m in range(2):
                        t1 = OQ.tile([128, T], f32, tag="ot1", name=f"ot1{m}")
                        nc.vector.tensor_scalar(out=t1[:], in0=opre[:, m, :],
                                                scalar1=osclB[:], scalar2=MAGIC,
                                                op0=OP.mult, op1=OP.add)
                        fin = OQ.tile([128, T], f32, tag="ofin", name=f"ofin{m}")
                        nc.vector.tensor_scalar(out=fin[:], in0=t1[:], scalar1=MAGIC,
                                                scalar2=oinvB[:], op0=OP.subtract,
                                                op1=OP.mult)
                        nc.sync.dma_start(o_outv[:, m, :], fin[:])

    nc.compile()
    return nc


def kernel(**inputs):
    import concourse.bass_utils as bass_utils

    x = np.asarray(inputs["x"], dtype=np.float32)
    bf = ml_dtypes.bfloat16
    xt = np.ascontiguousarray(x.reshape(T, DIM).T)            # [DIM, T]
    xhi = xt.astype(bf)
    r = xt - xhi.astype(np.float32)
    xlo = r.astype(bf)
    xlo2 = (r - xlo.astype(np.float32)).astype(bf)

    if "nc" not in _cache:
        _cache["nc"] = _build()
    nc = _cache["nc"]

    in_maps = []
    for c in range(NCORES):
        m = {"xhi": xhi, "xlo": xlo, "xlo2": xlo2}
        for p in "qkvo":
            w = np.asarray(inputs[f"w{p}"], dtype=np.float32)
            m[f"w{p}"] = np.ascontiguousarray(w[c * CH:(c + 1) * CH, :].T)
            m[f"b{p}"] = np.ascontiguousarray(
                np.asarray(inputs[f"b{p}"], dtype=np.float32)[c * CH:(c + 1) * CH])
            m[f"s{p}"] = np.asarray(inputs[f"s{p}"],
                                    dtype=np.float32).reshape(1, 1).copy()
        in_maps.append(m)

    res = bass_utils.run_bass_kernel_spmd(nc, in_maps, core_ids=list(range(NCORES)))
    full_T = np.concatenate([res.results[c]["o_out"] for c in range(NCORES)], axis=0)
    return np.ascontiguousarray(full_T.T).reshape(B, S, DIM).astype(np.float32)


if __name__ == "__main__":
    d = np.load("/root/problem/inputs_cache.npz")
    out = kernel(**{k: d[k] for k in d.files})
    ref = np.load("/root/problem/ref_out_np64.npy")
    err = np.linalg.norm((out - ref).ravel()) / np.linalg.norm(ref.ravel())
    print("Relative error vs fp64 ref:", err)



# revision 56
# speedup vs baseline: 1.4725x; 1.4725x over previous
"""BitAttention TRN2 kernel: 8-core tensor-parallel (head-split), v2.

Sharding: core c owns heads (2c, 2c+1) = channels [256c, 256c+256) of the
q/k/v projections (column split) and of the output channels of out_proj
(column split).  Attention is fully local to a core; one AllGather of the
(fp16) attention output feeds the out-projection.  The three global
reductions in the quantizers (mean|w|, max/min of q/k/v pre-acts, max/min
of out_proj pre-acts) are tiny AllReduces.

v2 changes vs v1 (cost-model-driven):
  - x split as an fp16 doublet (hi + 2^-11-scaled lo, ~22 mantissa bits)
    instead of a bf16 triplet: QKV projection is 2 PE passes instead of 3.
    The lo pass uses a 2^-11-scaled copy of the ternary weights so the
    pre-scale cancels inside the PE (fp16 represents +-2^-11 exactly).
  - softmax restructured to a single [q,k] score pass: DVE row-max feeds
    exp as a per-partition ACT bias, ACT accum_out produces the denominator
    for free, and the fp16 exp matrix is transposed to [k,q] by the DMA
    XBAR (dma_start_transpose) instead of recomputing scores transposed.
    Removes the 2nd score pass, the rank-1 max fold, and the denominator
    matmuls.
  - attention output allgathered as fp16 single precision (error budget
    verified by simulation), so the out-projection is 1 fp16 pass instead
    of 3 bf16 passes, and the allgather payload drops 3x.
  - allgather is split per batch and overlapped with attention of the
    other batch; out-projection consumes chunks as they arrive.
  - quantization round (MAGIC add/sub) moved to the ACT engine; DVE keeps
    only the reductions.
"""

import numpy as np
import ml_dtypes

DIM = 2048
NCORES = 8
CH = DIM // NCORES          # 256 channels per core
B, S = 2, 2048
T = B * S                   # 4096 tokens
KC = DIM // 128             # 16 contraction chunks
TT = 512                    # token tile
NTT = T // TT
MAGIC = float(1.5 * 2 ** 23)      # fp32 round-to-nearest-even via add/sub
F32MAX = float(np.finfo(np.float32).max)
LOSC = 2048.0                     # 2^11 lo-pass prescale

_cache = {}


def _build(single=False, stop_after=None):
    import concourse.bass as bass  # noqa: F401
    import concourse.mybir as mybir
    import concourse.tile as tile
    from concourse import bacc
    from concourse.bass_isa import ReduceOp
    from concourse.masks import make_identity

    f32 = mybir.dt.float32
    f32r = mybir.dt.float32r
    bf16 = mybir.dt.bfloat16
    f16 = mybir.dt.float16
    AX = mybir.AxisListType.X
    OP = mybir.AluOpType
    AF = mybir.ActivationFunctionType

    _ORDER = ["W", "Q", "C2", "S", "G", "O"]

    def _go(ph):
        return stop_after is None or _ORDER.index(ph) <= _ORDER.index(stop_after)

    nc = bacc.Bacc("TRN2", target_bir_lowering=False, debug=False,
                   num_devices=1 if single else NCORES)

    def collective(kind, op, in_ap, out_ap):
        if single:
            if kind == "AllGather":
                rows = in_ap.shape[0]
                for r in range(NCORES):
                    nc.sync.dma_start(out_ap[r * rows:(r + 1) * rows], in_ap)
            else:
                nc.scalar.dma_start(out_ap, in_ap)
        else:
            nc.gpsimd.collective_compute(kind, op, replica_groups=[list(range(NCORES))],
                                         ins=[in_ap.opt()], outs=[out_ap.opt()])

    def nrecip(pool, out_ap, d_ap, nm, shape=None):
        """out = 1/d with one Newton refinement on top of DVE reciprocal."""
        shape = shape or [d_ap.shape[0], d_ap.shape[-1]]
        g0 = pool.tile(shape, f32, tag=f"nr0_{shape[-1]}", name=f"g0_{nm}")
        t = pool.tile(shape, f32, tag=f"nr1_{shape[-1]}", name=f"t_{nm}")
        u = pool.tile(shape, f32, tag=f"nr2_{shape[-1]}", name=f"u_{nm}")
        nc.vector.reciprocal(g0[:], d_ap)
        nc.vector.tensor_tensor(out=t[:], in0=d_ap, in1=g0[:], op=OP.mult)
        nc.vector.tensor_scalar(out=t[:], in0=t[:], scalar1=1.0, scalar2=None,
                                op0=OP.subtract)
        nc.vector.tensor_tensor(out=u[:], in0=g0[:], in1=t[:], op=OP.mult)
        nc.vector.tensor_tensor(out=out_ap, in0=g0[:], in1=u[:], op=OP.subtract)

    # ---------------- I/O ----------------
    xhi = nc.dram_tensor("xhi", [DIM, T], f16, kind="ExternalInput").ap()
    xlo = nc.dram_tensor("xlo", [DIM, T], f16, kind="ExternalInput").ap()
    wT = {p: nc.dram_tensor(f"w{p}", [DIM, CH], f32, kind="ExternalInput").ap()
          for p in "qkvo"}
    bias = {p: nc.dram_tensor(f"b{p}", [CH], f32, kind="ExternalInput").ap()
            for p in "qkvo"}
    sca = {p: nc.dram_tensor(f"s{p}", [1, 1], f32, kind="ExternalInput").ap()
           for p in "qkvo"}
    o_out = nc.dram_tensor("o_out", [CH, T], f32, kind="ExternalOutput").ap()

    xv = {0: xhi.rearrange("(c p) t -> p c t", p=128),
          1: xlo.rearrange("(c p) t -> p c t", p=128)}
    wTv = {p: wT[p].rearrange("(c p) o -> p c o", p=128) for p in "qkvo"}
    bv = {p: bias[p].rearrange("(m p) -> p m", p=128) for p in "qkvo"}
    o_outv = o_out.rearrange("(m p) t -> p m t", p=128)

    with tile.TileContext(nc) as tc:
        with tc.tile_pool(name="persist", bufs=1) as P, \
             tc.tile_pool(name="dram", bufs=1, space="DRAM") as D:

            # ---- persistent arenas ----
            nqT = P.tile([128, 2, T], f16, name="nqT")        # [d, head, tok]
            nkT = P.tile([128, 2, T], f16, name="nkT")
            n_v = P.tile([128, 2, 2, KC, 128], f16, name="n_v")  # [kp,h,b,kc,ch]
            ident32 = P.tile([128, 128], f32, name="ident32")
            scal = P.tile([1, 16], f32, name="scal")          # partition-0 scalars
            scalB = P.tile([128, 4], f32, name="scalB")       # broadcast scalars
            stat_q = P.tile([128, 8], f32, name="stat_q")     # qkv+o max/negmin
            wsum = P.tile([128, 4], f32, name="wsum")
            wsum2 = P.tile([128, 4], f32, name="wsum2")
            magicB = P.tile([128, 1], f32, name="magicB")
            nmagicB = P.tile([128, 1], f32, name="nmagicB")

            make_identity(nc, ident32[:])
            nc.vector.memset(stat_q[:], -F32MAX)
            nc.vector.memset(magicB[:], MAGIC)
            nc.vector.memset(nmagicB[:], -MAGIC)

            # ---- dram scratch ----
            pre_d = {p: D.tile([2, 128, T], f32, name=f"pre_{p}") for p in "qkv"}
            cc1_in = D.tile([1, 4], f32, name="cc1_in")
            cc1_out = D.tile([1, 4], f32, name="cc1_out", addr_space="Shared")
            cc2_in = D.tile([1, 6], f32, name="cc2_in")
            cc2_out = D.tile([1, 6], f32, name="cc2_out", addr_space="Shared")
            cc3_in = D.tile([1, 2], f32, name="cc3_in")
            cc3_out = D.tile([1, 2], f32, name="cc3_out", addr_space="Shared")
            # contiguous per (batch, sub-chunk): collectives need contiguous APs
            ag_in = D.tile([2, 4, CH, 512], f16, name="ag_in")
            # one Shared tensor per allgather sub-chunk (single-writer rule);
            # each out-projection token tile consumes exactly one sub-chunk
            ag_out = [[D.tile([NCORES * CH, 512], f16, name=f"ag_out{b}_{sc}",
                              addr_space="Local" if single else "Shared")
                       for sc in range(4)] for b in range(2)]
            wo_d = D.tile([128, KC, CH], f16, name="wo_d")    # wo parked W->O

            with tc.tile_pool(name="wter", bufs=1) as WT:
                w_hi = {p: WT.tile([128, KC, CH], f16, name=f"wter_{p}")
                        for p in "qkv"}
                w_hi["o"] = WT.tile([128, KC, CH], f16, name="wter_o")
                w_lo = {p: WT.tile([128, KC, CH], f16, name=f"wlo_{p}")
                        for p in "qkv"}

                # ============ Phase W: weight ternarization ============
                # q,k on DVE; v,o on gpsimd so the two halves run in parallel.
                with tc.tile_pool(name="wstage", bufs=1) as WS:
                    s_b = WS.tile([128, 4], f32, name="s_b")
                    for pi, p in enumerate("qkvo"):
                        s_sb = WS.tile([1, 1], f32, tag="ssb", bufs=4,
                                       name=f"ssb_{p}")
                        nc.sync.dma_start(s_sb[:], sca[p])
                        nc.gpsimd.partition_broadcast(s_b[:, pi:pi + 1], s_sb[:])
                    ws = {}          # cached w*s per matrix (in the load tile)
                    for pi, p in enumerate("qkvo"):
                        eng = nc.vector
                        if p == "o":
                            wt = WS.tile([128, KC, CH], f32, tag="wf",
                                         name="wf_o")
                        else:
                            wt = WS.tile([128, KC, CH], f32, name=f"ws_{p}")
                        nc.sync.dma_start(wt[:], wTv[p])
                        eng.tensor_scalar_mul(wt[:], wt[:], s_b[:, pi:pi + 1])
                        ws[p] = wt
                        wl1 = WS.tile([128, KC], f32, tag=f"wl1{pi % 2}",
                                      name=f"wl1_{p}")
                        nc.vector.tensor_reduce(
                            out=wl1[:], in_=wt[:],
                            axis=AX, op=OP.add, apply_absolute_value=True)
                        nc.vector.tensor_reduce(
                            out=wsum[:, pi:pi + 1], in_=wl1[:],
                            axis=AX, op=OP.add)
                    nc.gpsimd.partition_all_reduce(wsum2[:], wsum[:], channels=128,
                                                   reduce_op=ReduceOp.add)
                    nc.scalar.dma_start(cc1_in[:], wsum2[0:1, 0:4])
                    collective("AllReduce", OP.add, cc1_in[:], cc1_out[:])
                    nc.scalar.dma_start(scal[:, 0:4], cc1_out[:])
                    thr = WS.tile([1, 4], f32, name="thr")
                    nthr = WS.tile([1, 4], f32, name="nthr")
                    nc.vector.tensor_scalar_mul(thr[:], scal[:, 0:4],
                                                0.7 / (DIM * DIM))
                    nc.vector.tensor_scalar_mul(nthr[:], thr[:], -1.0)
                    thr_b = WS.tile([128, 4], f32, name="thr_b")
                    nthr_b = WS.tile([128, 4], f32, name="nthr_b")
                    nc.gpsimd.partition_broadcast(thr_b[:], thr[:])
                    nc.gpsimd.partition_broadcast(nthr_b[:], nthr[:])
                    for pi, p in enumerate("qkvo"):
                        eng = nc.vector
                        if p == "o":
                            # o's w*s tile was a temp: reload and recompute
                            wt = WS.tile([128, KC, CH], f32, tag="wf",
                                         name="wf_o2")
                            nc.sync.dma_start(wt[:], wTv[p])
                            eng.tensor_scalar_mul(wt[:], wt[:], s_b[:, pi:pi + 1])
                            ws[p] = wt
                        gt = WS.tile([128, KC, CH], f16, tag=f"gt{pi % 2}",
                                     name=f"gt_{p}")
                        eng.tensor_scalar(out=gt[:], in0=ws[p][:],
                                          scalar1=thr_b[:, pi:pi + 1],
                                          scalar2=None, op0=OP.is_gt)
                        # (w*s < -thr) in place over the cached w*s (last use)
                        eng.tensor_scalar(out=ws[p][:], in0=ws[p][:],
                                          scalar1=nthr_b[:, pi:pi + 1],
                                          scalar2=None, op0=OP.is_lt)
                        eng.tensor_tensor(out=w_hi[p][:], in0=gt[:],
                                          in1=ws[p][:], op=OP.subtract)
                        if p != "o":
                            eng.tensor_scalar_mul(w_lo[p][:], w_hi[p][:],
                                                  1.0 / LOSC)
                    # park wo in DRAM until phase O (frees SBUF for phase Q)
                    nc.scalar.dma_start(wo_d[:], w_hi["o"][:])

                # ============ Phase Q: QKV projections (f32r single pass) ====
                _doQ = _go("Q")
                with tc.tile_pool(name="xstage", bufs=2) as XS, \
                     tc.tile_pool(name="qpsum", bufs=4, space="PSUM") as QP, \
                     tc.tile_pool(name="qout", bufs=3) as QO:
                    bsb = QO.tile([128, 3, 2], f32, bufs=1, name="bsb")
                    for pi, p in enumerate("qkv"):
                        nc.sync.dma_start(bsb[:, pi, :], bv[p])
                    for tt in range(NTT if _doQ else 0):
                        xs = {}
                        for pas in range(2):
                            xt = XS.tile([128, KC, TT], f16, tag=f"x{pas}",
                                         name=f"x{pas}_{tt}")
                            nc.sync.dma_start(
                                xt[:], xv[pas][:, :, tt * TT:(tt + 1) * TT])
                            xs[pas] = xt
                        for pi, p in enumerate("qkv"):
                            for m in range(2):
                                ps = QP.tile([128, TT], f32, tag="qp",
                                             name=f"qp{p}{m}{tt}")
                                n = 0
                                for kc in range(KC):
                                    for pas, wd in ((0, w_hi), (1, w_lo)):
                                        nc.tensor.matmul(
                                            ps[:],
                                            wd[p][:, kc, m * 128:(m + 1) * 128],
                                            xs[pas][:, kc, :],
                                            start=(n == 0), stop=(n == 2 * KC - 1))
                                        n += 1
                                pre = QO.tile([128, TT], f32, tag="pre",
                                              name=f"pre{p}{m}{tt}")
                                nc.scalar.activation(pre[:], ps[:], AF.Identity,
                                                     bias=bsb[:, pi, m:m + 1],
                                                     scale=1.0)
                                six = 2 * pi
                                tmx = QO.tile([128, 2], f32, tag="tmx",
                                              name=f"tmx{p}{m}{tt}")
                                nc.vector.tensor_reduce(out=tmx[:, 0:1], in_=pre[:],
                                                        axis=AX, op=OP.max)
                                nc.vector.tensor_reduce(out=tmx[:, 1:2], in_=pre[:],
                                                        axis=AX, op=OP.min,
                                                        negate=True)
                                nc.vector.tensor_tensor(out=stat_q[:, six:six + 2],
                                                        in0=stat_q[:, six:six + 2],
                                                        in1=tmx[:], op=OP.max)
                                nc.scalar.dma_start(
                                    pre_d[p][m, :, tt * TT:(tt + 1) * TT], pre[:])

            _doC2 = _go("C2")
            # ---- global max/min AllReduce + quantize q/k/v ----
            stat2 = P.tile([128, 6], f32, name="stat2")
            nc.gpsimd.partition_all_reduce(stat2[:], stat_q[:, 0:6], channels=128,
                                           reduce_op=ReduceOp.max)
            nc.scalar.dma_start(cc2_in[:], stat2[0:1, 0:6])
            collective("AllReduce", OP.max, cc2_in[:], cc2_out[:])
            nc.scalar.dma_start(scal[:, 4:10], cc2_out[:])
            with tc.tile_pool(name="qquant", bufs=2) as QQ:
                nvT = QQ.tile([128, 2, T], f16, bufs=1, name="nvT")
                scl = QQ.tile([1, 3], f32, bufs=1, name="scl")
                for pi in range(3):
                    df = QQ.tile([1, 1], f32, tag="df", name=f"df{pi}")
                    nc.vector.tensor_tensor(out=df[:],
                                            in0=scal[:, 4 + 2 * pi:5 + 2 * pi],
                                            in1=scal[:, 5 + 2 * pi:6 + 2 * pi],
                                            op=OP.add)
                    rcp = QQ.tile([1, 1], f32, tag="rcp", name=f"rcp{pi}")
                    nrecip(QQ, rcp[:], df[:], f"rscl{pi}")
                    nc.vector.tensor_scalar_mul(scl[:, pi:pi + 1], rcp[:], 255.0)
                sclB = QQ.tile([128, 3], f32, bufs=1, name="sclB")
                nc.gpsimd.partition_broadcast(sclB[:], scl[:])
                # Dexp = 1/(scale_q*scale_k*sqrt(128)); scalB: [Dexp, s_v, -Dexp]
                tmp = QQ.tile([1, 1], f32, bufs=1, name="tmpd")
                nc.vector.tensor_tensor(out=tmp[:], in0=scl[:, 0:1], in1=scl[:, 1:2],
                                        op=OP.mult)
                nc.vector.tensor_scalar_mul(tmp[:], tmp[:], float(np.sqrt(128.0)))
                dexp = QQ.tile([1, 1], f32, bufs=1, name="dexp")
                nrecip(QQ, dexp[:], tmp[:], "rdexp")
                ndexp = QQ.tile([1, 1], f32, bufs=1, name="ndexp")
                nc.vector.tensor_scalar_mul(ndexp[:], dexp[:], -1.0)
                nc.gpsimd.partition_broadcast(scalB[:, 0:1], dexp[:])
                nc.gpsimd.partition_broadcast(scalB[:, 1:2], scl[:, 2:3])
                nc.gpsimd.partition_broadcast(scalB[:, 2:3], ndexp[:])
                for pi, p in enumerate("qkv" if _doC2 else ""):
                    for m in range(2):
                        st = QQ.tile([128, T], f32, tag="qst", name=f"qst{p}{m}")
                        nc.sync.dma_start(st[:], pre_d[p][m])
                        t1 = QQ.tile([128, T], f32, tag="qt1", name=f"qt1{p}{m}")
                        # round(pre*scl): MAGIC add/sub, both on ACT
                        nc.scalar.activation(t1[:], st[:], AF.Identity,
                                             bias=magicB[:, 0:1],
                                             scale=sclB[:, pi:pi + 1])
                        dst = {"q": nqT, "k": nkT, "v": nvT}[p]
                        nc.scalar.activation(dst[:, m, :], t1[:], AF.Identity,
                                             bias=nmagicB[:, 0:1], scale=1.0)
                if _doC2:
                    for h in range(2):
                        for b in range(2):
                            nc.scalar.dma_start_transpose(
                                n_v[:, h, b], nvT[:, h, b * S:(b + 1) * S])

            # ============ Phase S: attention, single [q,k] pass ============
            _doS = _go("S")
            with tc.tile_pool(name="attnarena", bufs=1) as AA, \
                 tc.tile_pool(name="sexp", bufs=1) as SE, \
                 tc.tile_pool(name="ssm", bufs=2) as SP, \
                 tc.tile_pool(name="spp", bufs=4, space="PSUM") as PP:
                if _doS:
                    aoutT = AA.tile([128, 2, T], f32, name="aoutT")

                def s_scores_qt(h, b, qt, den):
                    """scores + row-max + exp + transpose for one 512-q chunk."""
                    asb = SE.tile([128, 4, S], f16, tag="asb", bufs=2,
                                  name=f"asb{h}{b}{qt}")
                    for qcl in range(4):
                        qc = qt * 4 + qcl
                        q0 = b * S + qc * 128
                        mx2 = SP.tile([128, 2], f32, tag="mx2",
                                      name=f"mx2_{h}{b}{qc}")
                        psl = []
                        for kh in range(2):
                            pss = PP.tile([128, 1024], f32, tag="pss",
                                          name=f"ss{h}{b}{qc}{kh}")
                            for kt in range(2):
                                k0 = b * S + kh * 1024 + kt * 512
                                nc.tensor.matmul(
                                    pss[:, kt * 512:(kt + 1) * 512],
                                    nqT[:, h, q0:q0 + 128],
                                    nkT[:, h, k0:k0 + 512],
                                    start=True, stop=True)
                            nc.vector.tensor_reduce(
                                out=mx2[:, kh:kh + 1], in_=pss[:],
                                axis=AX, op=OP.max)
                            psl.append(pss)
                        nbias = SP.tile([128, 1], f32, tag="nbias",
                                        name=f"nb{h}{b}{qc}")
                        # (-Dexp*mx0) min (-Dexp*mx1) == -Dexp*max(mx0,mx1);
                        # fused + inline on DVE: no cross-engine roundtrip
                        mxs = SP.tile([128, 1], f32, tag="mxs",
                                      name=f"mxs{h}{b}{qc}")
                        nc.vector.tensor_scalar_mul(mxs[:], mx2[:, 1:2],
                                                    scalB[:, 2:3])
                        nc.vector.scalar_tensor_tensor(
                            out=nbias[:], in0=mx2[:, 0:1],
                            scalar=scalB[:, 2:3], in1=mxs[:],
                            op0=OP.mult, op1=OP.min)
                        den2 = SP.tile([128, 2], f32, tag="den2",
                                       name=f"den2{h}{b}{qc}")
                        for kh in range(2):
                            nc.scalar.activation(
                                asb[:, qcl, kh * 1024:(kh + 1) * 1024],
                                psl[kh][:], AF.Exp, bias=nbias[:, 0:1],
                                scale=scalB[:, 0:1],
                                accum_out=den2[:, kh:kh + 1])
                        nc.vector.tensor_tensor(out=den[:, qc:qc + 1],
                                                in0=den2[:, 0:1],
                                                in1=den2[:, 1:2], op=OP.add)
                    atT = SE.tile([128, KC, 4, 128], f16, tag="atT",
                                  bufs=2, name=f"atT{h}{b}{qt}")
                    for qcl in range(4):
                        nc.scalar.dma_start_transpose(
                            atT[:, :, qcl, :], asb[:, qcl, :])
                    return atT

                def s_av_qt(h, b, qt, atT):
                    """attn @ v for one 512-q chunk (unnormalized)."""
                    po = PP.tile([128, 1024], f32, tag="pss",
                                 name=f"po{h}{b}{qt}")
                    for kc in range(KC):
                        nc.tensor.matmul(po[:, 0:512], n_v[:, h, b, kc, :],
                                         atT[:, kc, :, :],
                                         start=(kc == 0), stop=(kc == KC - 1))
                    qs = b * S + qt * 512
                    nc.scalar.copy(aoutT[:, h, qs:qs + 512], po[:, 0:512])

                def s_den_finish(h, b, den):
                    """deferred denominator division + AllGather chunk."""
                    dsb = SP.tile([128, 16], f32, tag="dsb", name=f"dsb{h}{b}")
                    nc.vector.tensor_scalar_mul(dsb[:], den[:], scalB[:, 1:2])
                    rden = SP.tile([128, 16], f32, tag="rden",
                                   name=f"rden{h}{b}")
                    nrecip(SP, rden[:], dsb[:], f"rg{h}{b}")
                    den_pad = SP.tile([128, 128], f32, tag="dpad",
                                      name=f"dpad{h}{b}")
                    nc.vector.memset(den_pad[:], 1.0)
                    nc.vector.tensor_copy(den_pad[:, 0:16], rden[:])
                    pnt = PP.tile([128, 1024], f32, tag="pss", name=f"pnt{h}{b}")
                    nc.tensor.transpose(pnt[:, 0:128], den_pad[:], ident32[:])
                    denT = SP.tile([16, 128], f32, tag="denT", name=f"denT{h}{b}")
                    nc.vector.tensor_copy(denT[:], pnt[0:16, 0:128])
                    grow = SP.tile([1, S], f32, tag="grow", name=f"grow{h}{b}")
                    nc.sync.dma_start(grow[:], denT[:])
                    gb = SP.tile([128, S], f32, tag="gb", name=f"gb{h}{b}")
                    nc.gpsimd.partition_broadcast(gb[:], grow[:])
                    bsl = slice(b * S, (b + 1) * S)
                    nc.vector.tensor_tensor(out=aoutT[:, h, bsl],
                                            in0=aoutT[:, h, bsl], in1=gb[:],
                                            op=OP.mult)
                    if h == 1 and _go("G"):
                        agh = SP.tile([128, 2, S], f16, tag="agh", bufs=1,
                                      name=f"agh{b}")
                        nc.vector.tensor_copy(agh[:],
                                              aoutT[:, :, b * S:(b + 1) * S])
                        # sub-chunked so the collective's DMA traffic
                        # interleaves with the attn transposes instead of
                        # monopolizing the DMA engines in one burst
                        for sc in range(4):
                            ssl = slice(sc * 512, (sc + 1) * 512)
                            agv = ag_in[b][sc].rearrange("(m p) s -> p m s",
                                                         p=128)
                            nc.sync.dma_start(agv, agh[:, :, ssl])
                            collective("AllGather", OP.bypass,
                                       ag_in[b][sc], ag_out[b][sc][:])

                # globally software-pipelined issue across all 16 chunks:
                # scores(c) before av(c-1), so a block's tail never drains
                # the pipeline; each block's den machinery issues two chunks
                # later, once its last av is already in flight.
                chunks = [(b, h, qt)
                          for b in range(2 if _doS else 0)
                          for h in range(2) for qt in range(4)]
                dens = {}
                prev = None
                for c in range(len(chunks) + 1):
                    if c < len(chunks):
                        b, h, qt = chunks[c]
                        if qt == 0:
                            dens[(h, b)] = SP.tile([128, 16], f32, tag="den",
                                                   bufs=2, name=f"den{h}{b}")
                        atT = s_scores_qt(h, b, qt, dens[(h, b)])
                    if prev is not None:
                        s_av_qt(*prev)
                    prev = (h, b, qt, atT) if c < len(chunks) else None
                    if c >= 6 and c % 4 == 2:
                        pb, ph, _ = chunks[c - 6]
                        s_den_finish(ph, pb, dens.pop((ph, pb)))
                if chunks:
                    for (ph, pb) in list(dens):
                        s_den_finish(ph, pb, dens.pop((ph, pb)))

            # ============ Phase O: out-projection (fp16 single) ============
            _doO = _go("O")
            with tc.tile_pool(name="oarena", bufs=1) as OA:
              if _doO:
                opre = OA.tile([128, 2, T], f32, name="opre")
                with tc.tile_pool(name="ostage", bufs=2) as OG, \
                     tc.tile_pool(name="opsum", bufs=4, space="PSUM") as OPP:
                    bosb = OG.tile([128, 2], f32, bufs=1, name="bosb")
                    nc.sync.dma_start(bosb[:], bv["o"])
                    wo_sb = OG.tile([128, KC, CH], f16, bufs=1, name="wo_sb")
                    nc.sync.dma_start(wo_sb[:], wo_d[:])
                    for tt in range(NTT if _doO else 0):
                        b = tt // 4
                        agov = ag_out[b][tt % 4][:].rearrange(
                            "(c p) s -> p c s", p=128)
                        ast = OG.tile([128, KC, TT], f16, tag="ast", name=f"ast{tt}")
                        nc.sync.dma_start(ast[:], agov)
                        for m in range(2):
                            ps = OPP.tile([128, TT], f32, tag="op", name=f"op{m}{tt}")
                            for kc in range(KC):
                                nc.tensor.matmul(
                                    ps[:], wo_sb[:, kc, m * 128:(m + 1) * 128],
                                    ast[:, kc, :],
                                    start=(kc == 0), stop=(kc == KC - 1))
                            nc.scalar.activation(opre[:, m, tt * TT:(tt + 1) * TT],
                                                 ps[:], AF.Identity,
                                                 bias=bosb[:, m:m + 1], scale=1.0)
                            tmx = OG.tile([128, 2], f32, tag="otmx",
                                          name=f"otmx{m}{tt}")
                            nc.vector.tensor_reduce(
                                out=tmx[:, 0:1],
                                in_=opre[:, m, tt * TT:(tt + 1) * TT],
                                axis=AX, op=OP.max)
                            nc.vector.tensor_reduce(
                                out=tmx[:, 1:2],
                                in_=opre[:, m, tt * TT:(tt + 1) * TT],
                                axis=AX, op=OP.min, negate=True)
                            nc.vector.tensor_tensor(out=stat_q[:, 6:8],
                                                    in0=stat_q[:, 6:8], in1=tmx[:],
                                                    op=OP.max)
                # ---- final quantization ----
                stat3 = P.tile([128, 2], f32, name="stat3")
                nc.gpsimd.partition_all_reduce(stat3[:], stat_q[:, 6:8],
                                               channels=128, reduce_op=ReduceOp.max)
                nc.scalar.dma_start(cc3_in[:], stat3[0:1, 0:2])
                collective("AllReduce", OP.max, cc3_in[:], cc3_out[:])
                nc.scalar.dma_start(scal[:, 10:12], cc3_out[:])
                with tc.tile_pool(name="oquant", bufs=2) as OQ:
                    df = OQ.tile([1, 1], f32, bufs=1, name="odf")
                    nc.vector.tensor_tensor(out=df[:], in0=scal[:, 10:11],
                                            in1=scal[:, 11:12], op=OP.add)
                    oscl = OQ.tile([1, 1], f32, bufs=1, name="oscl")
                    orcp = OQ.tile([1, 1], f32, bufs=1, name="orcp")
                    nrecip(OQ, orcp[:], df[:], "rorcp")
                    nc.vector.tensor_scalar_mul(oscl[:], orcp[:], 255.0)
                    oinv = OQ.tile([1, 1], f32, bufs=1, name="oinv")
                    nrecip(OQ, oinv[:], oscl[:], "roinv")
                    osclB = OQ.tile([128, 1], f32, bufs=1, name="osclB")
                    oinvB = OQ.tile([128, 1], f32, bufs=1, name="oinvB")
                    nc.gpsimd.partition_broadcast(osclB[:], oscl[:])
                    nc.gpsimd.partition_broadcast(oinvB[:], oinv[:])
                    for m in range(2):
                        t1 = OQ.tile([128, T], f32, tag="ot1", name=f"ot1{m}")
                        nc.vector.tensor_scalar(out=t1[:], in0=opre[:, m, :],
                                                scalar1=osclB[:], scalar2=MAGIC,
                                                op0=OP.mult, op1=OP.add)
                        fin = OQ.tile([128, T], f32, tag="ofin", name=f"ofin{m}")
                        nc.vector.tensor_scalar(out=fin[:], in0=t1[:], scalar1=MAGIC,
                                                scalar2=oinvB[:], op0=OP.subtract,
                                                op1=OP.mult)
                        nc.sync.dma_start(o_outv[:, m, :], fin[:])

    nc.compile()
    return nc


def kernel(**inputs):
    import concourse.bass_utils as bass_utils

    x = np.asarray(inputs["x"], dtype=np.float32)
    xt = np.ascontiguousarray(x.reshape(T, DIM).T)            # [DIM, T]
    xhi = xt.astype(np.float16)
    xlo = ((xt - xhi.astype(np.float32)) * np.float32(LOSC)).astype(np.float16)

    if "nc" not in _cache:
        _cache["nc"] = _build()
    nc = _cache["nc"]

    in_maps = []
    for c in range(NCORES):
        m = {"xhi": xhi, "xlo": xlo}
        for p in "qkvo":
            w = np.asarray(inputs[f"w{p}"], dtype=np.float32)
            m[f"w{p}"] = np.ascontiguousarray(w[c * CH:(c + 1) * CH, :].T)
            m[f"b{p}"] = np.ascontiguousarray(
                np.asarray(inputs[f"b{p}"], dtype=np.float32)[c * CH:(c + 1) * CH])
            m[f"s{p}"] = np.asarray(inputs[f"s{p}"],
                                    dtype=np.float32).reshape(1, 1).copy()
        in_maps.append(m)

    res = bass_utils.run_bass_kernel_spmd(nc, in_maps, core_ids=list(range(NCORES)))
    full_T = np.concatenate([res.results[c]["o_out"] for c in range(NCORES)], axis=0)
    return np.ascontiguousarray(full_T.T).reshape(B, S, DIM).astype(np.float32)


if __name__ == "__main__":
    d = np.load("/root/problem/inputs_cache.npz")
    out = kernel(**{k: d[k] for k in d.files})
    ref = np.load("/root/problem/ref_out_f32.npy")
    err = np.linalg.norm((out - ref).ravel()) / np.linalg.norm(ref.ravel())
    print("Relative error vs fp32 ref:", err)
